# revision 12
# baseline (speedup 1.0000x reference)
"""Trainium2 Bass kernel: 3D BFP activation quantization (shared-exponent blocks
of blk=16 contiguous channels along C), data-parallel over batch N across 8
NeuronCores.

kernel(activations[8,64,32,64,64] f32, mantissa=7, blk=16) -> same-shape f32.

Math (exact fp32/int32 bit arithmetic; matches the jnp reference bit-for-bit):
  per block b, spatial s:  M = max_i |x[blk*b+i, s]|
  E  = exponent field of M;  quantum = 2^(E-127-(mant-1));  scale = 1/quantum
  y   = x * scale                                  # exact (pow2), |y| < 2^mant
  t   = min(y + 1.5*2^23, 1.5*2^23 + lim)          # RNE round to int + high clip
  a   = relu(t - (1.5*2^23 - lim))                 # low clip; a = clipped + lim
  out = (a - lim) * quantum                        # exact

Bit manipulation (int32; Ei/Si/Qi are the exponent/scale/quantum bit patterns):
  Ei = max(Mbits & 0x7F800000, 0x0C800000)         # clamp E>=25 so zero blocks
                                                   #   get finite scale/quantum
  Si = 0x82000000 - Ei  (= ~Ei + 0x82000001)       # scale = 2^(133-E)
  Qi = Ei - ((mant-1)<<23)                         # quantum = 2^(E-133)

bf16 tail: a in [0, 2*lim] and (a-lim) in [-lim, lim] are small integers and
quantum is a power of two, so relu -> bf16, the (a-lim)*quantum stt in bf16,
and the bf16->f32 cast during the store DMA are all exact. This halves DVE
cost of the stt (2x_1p mode) and keeps the output bit-identical.

Layout: partition = 128 spatial positions, free = (16 channels = 1 block,
F=512 spatial). DRAM rows per partition are 2KB contiguous -> efficient DMA.
"""

import os
import sys

for _p in ("/opt/trn_rl_repo", "/root/.axon_site/_ro/trn_rl_repo"):
    if os.path.isdir(_p) and _p not in sys.path:
        sys.path.insert(0, _p)

import numpy as np

# ---- hardcoded problem geometry ----
N, C, D, H, W = 8, 64, 32, 64, 64
S = D * H * W                 # 131072 spatial per (n, c)
N_CORES = 8
PD = 128                      # SBUF partitions (spatial)

_BUILT = {}


def _build(mant: int, blk: int, F: int = 512, CT: int = 16, bufs: int = 4,
           abufs: int = 3, mbufs: int = 3,
           mult_engine: str = "gpsimd", relu_engine: str = "scalar",
           use_bf16: int = 1, store_engine: str = "gpsimd",
           C: int = C, S: int = S, mode: str = "full", repeat: int = 1):
    import concourse.bass as bass
    import concourse.bacc as bacc
    import concourse.mybir as mybir
    from concourse.tile import TileContext

    FP32 = mybir.dt.float32
    BF16 = mybir.dt.bfloat16
    I32 = mybir.dt.int32
    Alu = mybir.AluOpType

    NBt = CT // blk           # channel blocks per tile
    NCC = C // CT             # channel chunks
    NTS = S // (PD * F)       # spatial chunks
    assert S % (PD * F) == 0 and C % CT == 0 and CT % blk == 0

    LIM = float(2 ** mant - 1)
    MAGIC = 1.5 * 2.0 ** 23
    EXP_OFF = (mant - 1) << 23

    nc = bacc.Bacc("TRN2", target_bir_lowering=False)
    x_d = nc.dram_tensor("x", [C, S], FP32, kind="ExternalInput")
    o_d = nc.dram_tensor("o", [C, S], FP32, kind="ExternalOutput")

    xr = x_d[:].rearrange("(cc ct) (ts sh f) -> ts cc sh ct f",
                          cc=NCC, ct=CT, ts=NTS, sh=PD, f=F)
    orr = o_d[:].rearrange("(cc ct) (ts sh f) -> ts cc sh ct f",
                           cc=NCC, ct=CT, ts=NTS, sh=PD, f=F)

    abufs = abufs or bufs
    mbufs = mbufs or bufs
    with TileContext(nc) as tc:
        with (
            tc.tile_pool(name="xp", bufs=bufs) as xp,
            tc.tile_pool(name="ap", bufs=abufs) as ap,
            tc.tile_pool(name="mp", bufs=mbufs) as mp,
            tc.tile_pool(name="qp", bufs=bufs) as qp,
            tc.tile_pool(name="cp", bufs=1) as cp,
        ):
            relu_bias = cp.tile([PD, 1], FP32, tag="rbias")
            nc.vector.memset(relu_bias[:], -(MAGIC - LIM))
            tl = [(ts, cc) for ts in range(NTS) for cc in range(NCC)] * repeat

            # Software-pipelined emission: engines execute their streams
            # in order, so per-tile sequential emission exposes the
            # mult(Pool) and relu(ACT) latencies as DVE stalls. Skewing
            # the stages two deep keeps every engine's queue dense:
            #   iter i: head(i) [reduce+smalls+mult], mid(i-1)
            #           [round+relu], tail(i-2) [ts+tt+store]
            staged = {}

            def issue_load(i):
                ts_, cc_ = tl[i]
                Xl = xp.tile([PD, CT, F], FP32, tag="x")
                nc.sync.dma_start(Xl[:], xr[ts_, cc_])
                staged[i] = [Xl, None, None]

            def head(i):
                X = staged[i][0]
                M = mp.tile([PD, NBt, F], FP32, tag="m")
                Q = mp.tile([PD, NBt, F], FP32, tag="q")
                Sc = mp.tile([PD, NBt, F], FP32, tag="s")
                Q16 = (qp.tile([PD, NBt, F], BF16, tag="q16", name="Q16")
                       if use_bf16 else None)
                # block abs-max over i: AP [p, b, f, i], innermost strided
                nc.vector.tensor_reduce(
                    M[:],
                    X[:].rearrange("p (b i) f -> p b f i", b=NBt, i=blk),
                    axis=mybir.AxisListType.X, op=Alu.max,
                    apply_absolute_value=True,
                )
                Mi = M[:].bitcast(I32)
                Qi = Q[:].bitcast(I32)
                Si = Sc[:].bitcast(I32)
                # op0/op1 must share an ALU class (bitwise vs arith), so 4 ops:
                # V = 0x7F800000 - Ebits   (pure bitwise complement trick)
                nc.vector.tensor_scalar(
                    Si, Mi, 0x807FFFFF - (1 << 32), -1,
                    op0=Alu.bitwise_or, op1=Alu.bitwise_xor,
                )
                # scale bits = min(V, 0x73000000) + 0x02800000
                nc.vector.tensor_scalar(
                    Si, Si, 0x73000000, 0x02800000,
                    op0=Alu.min, op1=Alu.add,
                )
                # Ebits
                nc.vector.tensor_scalar(
                    Qi, Mi, 0x7F800000, None, op0=Alu.bitwise_and
                )
                # quantum bits = max(Ebits, 0x0C800000) - ((mant-1)<<23)
                nc.vector.tensor_scalar(
                    Qi, Qi, 0x0C800000, EXP_OFF,
                    op0=Alu.max, op1=Alu.subtract,
                )
                if use_bf16:
                    # quantum as bf16 (exact: power of two)
                    nc.scalar.activation(
                        Q16[:], Q[:], mybir.ActivationFunctionType.Copy,
                        bias=0.0, scale=1.0,
                    )
                X4 = X[:].rearrange("p (b i) f -> p b i f", b=NBt, i=blk)
                Sb = Sc[:].unsqueeze(2).broadcast_to([PD, NBt, blk, F])
                # y = x * scale  (exact pow2 mult)
                getattr(nc, mult_engine).tensor_tensor(X4, X4, Sb, op=Alu.mult)
                staged[i][2] = Q16 if use_bf16 else Q

            def mid(i):
                X = staged[i][0]
                X2 = X[:].rearrange("p c f -> p (c f)")
                # t = min(y + MAGIC, MAGIC + lim): RNE round + high clip
                nc.vector.tensor_scalar(
                    X2, X2, MAGIC, MAGIC + LIM, op0=Alu.add, op1=Alu.min
                )
                # a = relu(t - (MAGIC - lim)): low clip; a in [0, 2*lim] ints
                if use_bf16:
                    A16 = ap.tile([PD, CT, F], BF16, tag="a")
                    A2 = A16[:].rearrange("p c f -> p (c f)")
                    nc.scalar.activation(
                        A2, X2, mybir.ActivationFunctionType.Relu,
                        bias=relu_bias[:], scale=1.0,
                    )
                    staged[i][1] = A16
                else:
                    if relu_engine == "scalar":
                        nc.scalar.activation(
                            X2, X2, mybir.ActivationFunctionType.Relu,
                            bias=relu_bias[:], scale=1.0,
                        )
                    else:
                        nc.vector.tensor_scalar(
                            X2, X2, MAGIC - LIM, 0.0,
                            op0=Alu.subtract, op1=Alu.max,
                        )
                    staged[i][1] = X

            def tail(i):
                ts_, cc_ = tl[i]
                X, A16, Qq = staged.pop(i)
                if use_bf16:
                    A2 = A16[:].rearrange("p c f -> p (c f)")
                    # out = (a - lim) * quantum, all bf16 (exact), in place.
                    # tensor_scalar (4x mode) + tensor_tensor (2x) instead
                    # of one scalar_tensor_tensor (1x only).
                    nc.vector.tensor_scalar(
                        A2, A2, LIM, None, op0=Alu.subtract,
                    )
                    for b in range(NBt):
                        A3 = A16[:, b * blk:(b + 1) * blk]
                        Qb = Qq[:, b].unsqueeze(1).broadcast_to(
                            [PD, blk, F])
                        nc.vector.tensor_tensor(A3, A3, Qb, op=Alu.mult)
                    # store with bf16 -> f32 cast (SWDGE)
                    nc.gpsimd.dma_start(orr[ts_, cc_], A16[:])
                else:
                    X4 = X[:].rearrange("p (b i) f -> p b i f", b=NBt, i=blk)
                    for b in range(NBt):
                        Qb = Qq[:, b].unsqueeze(1).broadcast_to([PD, blk, F])
                        nc.vector.scalar_tensor_tensor(
                            X4[:, b], X4[:, b], LIM, Qb,
                            op0=Alu.subtract, op1=Alu.mult,
                        )
                    nc.sync.dma_start(orr[ts_, cc_], X[:])

            n = len(tl)
            for i in range(min(bufs, n)):
                issue_load(i)
            if mode == "copy":
                for i, (ts_, cc_) in enumerate(tl):
                    X = staged.pop(i)[0]
                    nc.sync.dma_start(orr[ts_, cc_], X[:])
                    if i + bufs < n:
                        issue_load(i + bufs)
            else:
                for i in range(n):
                    head(i)
                    if i >= 1:
                        mid(i - 1)
                    if i >= 2:
                        tail(i - 2)
                    if i + bufs < n:
                        issue_load(i + bufs)
                if n >= 1:
                    mid(n - 1)
                if n >= 2:
                    tail(n - 2)
                if n >= 1:
                    tail(n - 1)
    return nc


def _get_nc(mant: int, blk: int, **kw):
    key = (mant, blk, tuple(sorted(kw.items())))
    if key not in _BUILT:
        nc = _build(mant, blk, **kw)
        if not nc.is_finalized():
            nc.finalize()
        _BUILT[key] = nc
    return _BUILT[key]


def kernel(activations, mantissa=7, blk=16, **_ignored):
    from concourse.bass_utils import run_bass_kernel_spmd

    mant = int(np.asarray(mantissa))
    blk = int(np.asarray(blk))
    x = np.asarray(activations, dtype=np.float32)
    assert x.shape == (N, C, D, H, W), x.shape
    assert blk == 16 and C % blk == 0

    nc = _get_nc(mant, blk)
    xf = x.reshape(N, C, S)
    in_maps = [{"x": np.ascontiguousarray(xf[n])} for n in range(N_CORES)]
    res = run_bass_kernel_spmd(nc, in_maps, list(range(N_CORES)))
    outs = [np.asarray(r["o"], dtype=np.float32) for r in res.results]
    return np.stack(outs, axis=0).reshape(N, C, D, H, W)
